# revision 19
# baseline (speedup 1.0000x reference)
"""GPT-2 style attention block (B=2, S=2048, D=1024, H=16) on 8 TRN2 NeuronCores.

Sharding: tensor-parallel over heads + data-parallel over batch.
Cores 0-3 handle batch 0, cores 4-7 handle batch 1; each core owns 4 of the
16 heads (its 256-column slice of the qkv projection and the matching
256-row slice of c_proj_w). Each core produces a partial output
[S, D] = ctx_heads @ c_proj_rows; the 4 partials per batch are summed on
the host to give that batch's output.

v2 design notes (vs the v1 baseline):
  - hs is transposed and cast to fp16 on the HOST, so the on-device
    PE-transpose phase (~60us incl. copies) is gone entirely.
  - every matmul runs in fp16 (fp32r "HIGH" mode matmuls measured ~950ns
    per 512-row stream vs ~760 for fp16 under throttle; fp16 also lowers
    PE power draw which drives the 50%-duty throttle windows).
  - causal trimming: for the 4 diagonal key-tiles of each query block the
    score matmuls / exp / AV matmuls are restricted to the valid column
    range; only the 128-wide boundary strip needs a (precomputed tril)
    mask multiply on DVE.
  - softmax denominator reciprocal via reciprocal_approx_fast (the exact
    DVE reciprocal on a [1,512] AP ran ~4us each, 64us total in v1).
  - emission order interleaves projection chunks, attention query-blocks
    and output-projection row-tiles so the ACT-engine-bound attention
    (exp is ~1.15us per [128,1024] tile, ~96us total) overlaps the
    PE-bound projections.

Per-core pipeline:
  1. Q^T/K^T = (W_qk^T stationary) @ hs^T  -> [512, S] head-major rows;
     V = (hs^T stationary) @ W_v -> [S, 256] natural layout, stored
     augmented with a ones column per head ([S, 4*65]).
  2. per head pair hp, per 512-wide query block qb, per causal k-tile kt:
       S^T[k,q] for both heads via row-group-packed matmuls (K=64 each,
       tile_position (0,0)/(64,0)) into one [128,1024] PSUM pair
       expS = exp(S^T/8) for both heads in one ACT op (scores are O(3),
       no max-subtraction needed), trimmed to the causally valid columns
       ctx_aug^T[65, q] += V_aug[k,:].T @ expS_h  (row 64 = softmax denom)
     then ctx^T = ctx_aug^T[0:64] * broadcast(approx_recip(denom))
  3. out_partial[q, :] = ctx^T.T @ W_p_rows

The bias rows (c_attn_b v-slice folded through c_proj_w, plus c_proj_b)
are added on the host during unsharding (they are exactly zero for the
reference setup_inputs). The causal_mask input is the deterministic tril
mask from setup_inputs(); causality is implemented analytically on
device, so the mask tensor itself is unused.
"""

import numpy as np

B, S, D, H = 2, 2048, 1024, 16
HD = D // H  # 64
N_CORES = 8
HPC = 4  # heads per core
GROUPS = 4  # cores per batch
HSL = HPC * HD  # 256: per-core head-column width

_nc_cache = {}


def _build():
    import concourse.bacc as bacc
    import concourse.mybir as mybir
    import concourse.tile as tile

    f32 = mybir.dt.float32
    f16 = mybir.dt.float16

    nc = bacc.Bacc("TRN2", debug=False, num_devices=N_CORES)

    hst = nc.dram_tensor("hst", [D, S], f16, kind="ExternalInput")
    wqkv = nc.dram_tensor("wqkv", [D, 3 * HSL], f16, kind="ExternalInput")
    wp = nc.dram_tensor("wp", [HSL, D], f16, kind="ExternalInput")
    bqk = nc.dram_tensor("bqk", [2 * HSL], f32, kind="ExternalInput")
    outp = nc.dram_tensor("outp", [S, D], f16, kind="ExternalOutput")

    NDT = D // 128  # 8 contraction tiles
    NQB = S // 512  # 4 query blocks
    NKT = S // 128  # 16 key tiles
    EXPSCALE = float(1.0 / np.sqrt(HD))
    EXP = mybir.ActivationFunctionType.Exp
    IDENT = mybir.ActivationFunctionType.Identity

    with tile.TileContext(nc) as tc:
        with (
            tc.tile_pool(name="persist", bufs=1) as persist,
            tc.tile_pool(name="es", bufs=3) as es_pool,
            tc.tile_pool(name="rb", bufs=4) as rb_pool,
            tc.tile_pool(name="ob", bufs=3) as ob_pool,
            tc.tile_pool(name="sc", bufs=2, space="PSUM") as sc_pool,
            tc.tile_pool(name="cx", bufs=2, space="PSUM") as cx_pool,
            tc.tile_pool(name="pj", bufs=2, space="PSUM") as pj_pool,
        ):
            hsT_t = [
                [
                    persist.tile([128, 512], f16, name=f"hsT{dt}_{nt}")
                    for nt in range(NQB)
                ]
                for dt in range(NDT)
            ]
            wqkv_t = [
                persist.tile([128, 3 * HSL], f16, name=f"wqkv{dt}")
                for dt in range(NDT)
            ]
            wp_sb = persist.tile([128, 2, D], f16)
            bqk_sb = persist.tile([128, 4], f32)
            qkT = persist.tile([128, 4, S], f16)  # ct: 0,1=Q h01/h23, 2,3=K
            vv = persist.tile([128, NKT, HPC * (HD + 1)], f16)  # V aug
            ctxT_t = [
                [
                    persist.tile([128, 512], f16, name=f"ctxT{hp}_{qb}")
                    for qb in range(NQB)
                ]
                for hp in range(2)
            ]
            tril = persist.tile([128, 128], f16)

            # causal boundary mask: keep where q - p >= 0
            nc.gpsimd.memset(tril, 1.0)
            nc.gpsimd.affine_select(
                out=tril,
                in_=tril,
                compare_op=mybir.AluOpType.is_ge,
                fill=0.0,
                base=0,
                pattern=[[1, 128]],
                channel_multiplier=-1,
            )
            ones_src = persist.tile([128, HPC, 1], f16)
            nc.vector.memset(ones_src, 1.0)
            for kt in range(NKT):
                vcols = vv[:, kt, :].rearrange("p (h c) -> p h c", c=HD + 1)
                nc.vector.tensor_copy(vcols[:, :, HD : HD + 1], ones_src)

            # input DMAs: per-dt weight/hs^T pairs so the first matmul's
            # operands are behind only 3 DMA issues (~2.5us), and later
            # dt tiles stream in behind it
            nc.sync.dma_start(out=bqk_sb, in_=bqk.rearrange("(t p) -> p t", p=128))
            for dt in range(NDT):
                nc.sync.dma_start(
                    out=wqkv_t[dt],
                    in_=wqkv[dt * 128 : (dt + 1) * 128, :],
                )
                nc.sync.dma_start(
                    out=hsT_t[dt][0],
                    in_=hst[dt * 128 : (dt + 1) * 128, 0:512],
                )
            for nt in range(1, NQB):
                for dt in range(NDT):
                    nc.sync.dma_start(
                        out=hsT_t[dt][nt],
                        in_=hst[
                            dt * 128 : (dt + 1) * 128, nt * 512 : (nt + 1) * 512
                        ],
                    )
            nc.sync.dma_start(out=wp_sb, in_=wp.rearrange("(t p) n -> p t n", p=128))

            def emit_proj_chunk(nt):
                # Q^T/K^T columns [512*nt, 512*(nt+1)) for all 4 ct tiles
                for ct in range(4):
                    pj = pj_pool.tile([128, 512], f32, tag="pj", name=f"pj{nt}_{ct}")
                    for dt in range(NDT):
                        nc.tensor.matmul(
                            pj,
                            wqkv_t[dt][:, ct * 128 : (ct + 1) * 128],
                            hsT_t[dt][nt],
                            start=(dt == 0),
                            stop=(dt == NDT - 1),
                        )
                    nc.scalar.activation(
                        qkT[:, ct, nt * 512 : (nt + 1) * 512],
                        pj,
                        IDENT,
                        bias=bqk_sb[:, ct : ct + 1],
                    )
                # V rows [512*nt, 512*(nt+1))
                for rt in range(4 * nt, 4 * nt + 4):
                    pvf = pj_pool.tile([128, 512], f32, tag="pj", name=f"pv{rt}")
                    pv = pvf[:, :HSL]
                    for dt in range(NDT):
                        nc.tensor.matmul(
                            pv,
                            hsT_t[dt][rt // 4][
                                :, (rt % 4) * 128 : (rt % 4 + 1) * 128
                            ],
                            wqkv_t[dt][:, 2 * HSL : 3 * HSL],
                            start=(dt == 0),
                            stop=(dt == NDT - 1),
                        )
                    vtgt = vv[:, rt, :].rearrange("p (h c) -> p h c", c=HD + 1)
                    nc.vector.tensor_copy(
                        vtgt[:, :, 0:HD], pv.rearrange("p (h c) -> p h c", c=HD)
                    )

            def emit_attn_qb(qb, hps=(0, 1)):
                kmax = 4 * (qb + 1)
                for hp in hps:
                    cxa = cx_pool.tile([65, 512], f32, tag="cx", name=f"cxa{qb}_{hp}")
                    cxb = cx_pool.tile([65, 512], f32, tag="cx", name=f"cxb{qb}_{hp}")
                    cxs = (cxa, cxb)

                    def emit_av(kt, es3, off):
                        for hh in range(2):
                            h = 2 * hp + hh
                            nc.tensor.matmul(
                                cxs[hh][:, off:512],
                                vv[:, kt, h * (HD + 1) : (h + 1) * (HD + 1)],
                                es3[:, hh, off:512],
                                start=(kt == 0),
                                stop=(kt == kmax - 1),
                                skip_group_check=True,
                            )

                    pend = None  # AV runs one k-tile behind scores/exp
                    for kt in range(kmax):
                        j = kt - 4 * qb  # >= 0 on the diagonal tiles
                        off = 128 * j if j > 0 else 0
                        scp = sc_pool.tile(
                            [128, 1024], f32, tag="sc", name=f"sc{qb}_{hp}_{kt}"
                        )
                        sc3 = scp.rearrange("p (h c) -> p h c", c=512)
                        for hh in range(2):
                            nc.tensor.matmul(
                                scp[:, hh * 512 + off : (hh + 1) * 512],
                                qkT[
                                    hh * 64 : (hh + 1) * 64,
                                    2 + hp,
                                    kt * 128 : (kt + 1) * 128,
                                ],
                                qkT[
                                    hh * 64 : (hh + 1) * 64,
                                    hp,
                                    qb * 512 + off : (qb + 1) * 512,
                                ],
                                start=True,
                                stop=True,
                                tile_position=(hh * 64, 0),
                            )
                        es = es_pool.tile([128, 1024], f16, tag="es")
                        es3 = es.rearrange("p (h c) -> p h c", c=512)
                        nc.scalar.activation(
                            es3[:, :, off:512],
                            sc3[:, :, off:512],
                            EXP,
                            scale=EXPSCALE,
                        )
                        if j >= 0:  # mask the 128-wide boundary strip
                            for hh in range(2):
                                nc.vector.tensor_mul(
                                    es3[:, hh, off : off + 128],
                                    es3[:, hh, off : off + 128],
                                    tril,
                                )
                        if pend is not None:
                            emit_av(*pend)
                        pend = (kt, es3, off)
                    emit_av(*pend)

                    for hh in range(2):
                        # reciprocal_approx_fast misreads PSUM sources on HW
                        # (integer-ALU seed path); stage the denom row first
                        dstage = rb_pool.tile([1, 512], f32, tag="dst")
                        nc.vector.tensor_copy(dstage, cxs[hh][64:65, :])
                        rec = rb_pool.tile([1, 512], f32, tag="rec")
                        nc.vector.reciprocal_approx_fast(rec, dstage)
                        rbt = rb_pool.tile([64, 512], f32, tag="rbt")
                        nc.gpsimd.partition_broadcast(rbt, rec)
                        nc.vector.tensor_mul(
                            ctxT_t[hp][qb][hh * 64 : (hh + 1) * 64, :],
                            cxs[hh][0:64, :],
                            rbt,
                        )

            def emit_outproj_qb(qb, mts=None):
                for mt in mts if mts is not None else range(4 * qb, 4 * qb + 4):
                    po0 = pj_pool.tile([128, 512], f32, tag="pj", name=f"po0_{mt}")
                    po1 = pj_pool.tile([128, 512], f32, tag="pj", name=f"po1_{mt}")
                    pos = (po0, po1)
                    for ht in range(2):
                        for et in range(2):
                            nc.tensor.matmul(
                                pos[et],
                                ctxT_t[ht][mt // 4][
                                    :, (mt % 4) * 128 : (mt % 4 + 1) * 128
                                ],
                                wp_sb[:, ht, et * 512 : (et + 1) * 512],
                                start=(ht == 0),
                                stop=(ht == 1),
                            )
                    ob = ob_pool.tile([128, 1024], f16, tag="ob")
                    nc.vector.tensor_copy(ob[:, 0:512], pos[0])
                    nc.vector.tensor_copy(ob[:, 512:1024], pos[1])
                    nc.sync.dma_start(
                        out=outp[mt * 128 : (mt + 1) * 128, :], in_=ob
                    )

            # emission order: every attention head-pair boundary gets a
            # filler chunk (projection or output-projection) so the PE never
            # idles on the cx-pool rotation + normalize chain between pairs
            emit_proj_chunk(0)
            emit_attn_qb(0, hps=(0,))
            emit_proj_chunk(1)
            emit_attn_qb(0, hps=(1,))
            emit_outproj_qb(0)
            emit_attn_qb(1, hps=(0,))
            emit_proj_chunk(2)
            emit_attn_qb(1, hps=(1,))
            emit_outproj_qb(1)
            emit_attn_qb(2, hps=(0,))
            emit_proj_chunk(3)
            emit_attn_qb(2, hps=(1,))
            emit_outproj_qb(2, mts=(8, 9))
            emit_attn_qb(3, hps=(0,))
            emit_outproj_qb(2, mts=(10, 11))
            emit_attn_qb(3, hps=(1,))
            emit_outproj_qb(3)

    nc.compile()
    return nc

def build_kernel(*_args, **_kwargs):
    if "k" not in _nc_cache:
        _nc_cache["k"] = _build()
    return _nc_cache["k"]


def make_in_maps(
    hidden_states, c_attn_w, c_attn_b, c_proj_w, c_proj_b, **_unused
):
    hidden_states = np.asarray(hidden_states, dtype=np.float32)
    c_attn_w = np.asarray(c_attn_w, dtype=np.float32)
    c_attn_b = np.asarray(c_attn_b, dtype=np.float32)
    c_proj_w = np.asarray(c_proj_w, dtype=np.float32)
    c_proj_b = np.asarray(c_proj_b, dtype=np.float32)

    in_maps = []
    for c in range(N_CORES):
        b, g = divmod(c, GROUPS)
        cs = slice(g * HSL, (g + 1) * HSL)
        wq = c_attn_w[:, g * HSL : (g + 1) * HSL]
        wk = c_attn_w[:, D + g * HSL : D + (g + 1) * HSL]
        wv = c_attn_w[:, 2 * D + g * HSL : 2 * D + (g + 1) * HSL]
        bq = c_attn_b[g * HSL : (g + 1) * HSL]
        bk = c_attn_b[D + g * HSL : D + (g + 1) * HSL]
        bv = c_attn_b[2 * D + g * HSL : 2 * D + (g + 1) * HSL]
        wps = c_proj_w[cs, :]
        rr = bv.astype(np.float64) @ wps.astype(np.float64)
        if g == 0:
            rr = rr + c_proj_b
        in_maps.append(
            {
                "hst": np.ascontiguousarray(
                    hidden_states[b].T.astype(np.float16)
                ),
                "wqkv": np.ascontiguousarray(
                    np.concatenate([wq, wk, wv], axis=1).astype(np.float16)
                ),
                "wp": np.ascontiguousarray(wps.astype(np.float16)),
                "bqk": np.ascontiguousarray(
                    np.concatenate([bq, bk]).astype(np.float32)
                ),
                "_rrow": np.ascontiguousarray(rr.astype(np.float32)),
            }
        )
    return in_maps


def kernel(
    hidden_states,
    c_attn_w,
    c_attn_b,
    c_proj_w,
    c_proj_b,
    causal_mask=None,
    **_unused,
):
    from concourse.bass_utils import run_bass_kernel_spmd

    nc = build_kernel()
    in_maps = make_in_maps(
        hidden_states, c_attn_w, c_attn_b, c_proj_w, c_proj_b
    )
    rrows = [m.pop("_rrow") for m in in_maps]
    res = run_bass_kernel_spmd(nc, in_maps, list(range(N_CORES)))
    out = np.zeros((B, S, D), dtype=np.float32)
    for c in range(N_CORES):
        out[c // GROUPS] += res.results[c]["outp"] + rrows[c]
    return out


# revision 20
# speedup vs baseline: 1.0528x; 1.0528x over previous
"""GPT-2 style attention block (B=2, S=2048, D=1024, H=16) on 8 TRN2 NeuronCores.

Sharding: tensor-parallel over heads + data-parallel over batch.
Cores 0-3 handle batch 0, cores 4-7 handle batch 1; each core owns 4 of the
16 heads (its 256-column slice of the qkv projection and the matching
256-row slice of c_proj_w). Each core produces a partial output
[S, D] = ctx_heads @ c_proj_rows; the 4 partials per batch are summed on
the host to give that batch's output.

Design notes (vs the v1 baseline, 349us -> ~168us):
  - hs is transposed and cast to fp16 on the HOST, so the on-device
    PE-transpose phase (~60us incl. copies) is gone entirely.
  - every matmul runs in fp16 (fp32r "HIGH" mode matmuls measured ~950ns
    per 512-row stream vs ~380 for fp16; fp16 also lowers PE power draw,
    cutting the 50%-duty hardware throttle windows from 155us to ~17us).
  - causal trimming: for the 4 diagonal key-tiles of each query block the
    score matmuls / exp / AV matmuls are restricted to the causally valid
    column range; only the 128-wide boundary strip needs a (precomputed
    tril) mask multiply on DVE.
  - softmax denominator reciprocal via reciprocal_approx_fast on a staged
    SBUF copy (the exact DVE reciprocal on a [1,512] AP ran ~4us each,
    64us total; reciprocal_approx_fast misreads PSUM sources on HW so the
    denom row is copied to SBUF first).
  - emission order interleaves projection chunks, attention query-blocks
    and output-projection row-tiles so the ACT-engine exp stream (~79us)
    overlaps the PE-bound projections; outproj chunks are placed so the
    tile-granular ctxT dep tracker never stalls them long on unrelated
    normalize writes.
  - fp16 partial output (summed in fp32 on the host) halves the output
    DMA drain at the kernel tail.

Per-core pipeline:
  1. Q^T/K^T = (W_qk^T stationary) @ hs^T  -> [512, S] head-major rows;
     V = (hs^T stationary) @ W_v -> [S, 256] natural layout, stored
     augmented with a ones column per head ([S, 4*65]).
  2. per head pair hp, per 512-wide query block qb, per causal k-tile kt:
       S^T[k,q] for both heads via row-group-packed matmuls (K=64 each,
       tile_position (0,0)/(64,0)) into one [128,1024] PSUM pair
       expS = exp(S^T/8) for both heads in one ACT op (scores are O(3),
       no max-subtraction needed), trimmed to the causally valid columns
       ctx_aug^T[65, q] += V_aug[k,:].T @ expS_h  (row 64 = softmax denom)
     then ctx^T = ctx_aug^T[0:64] * broadcast(approx_recip(denom))
  3. out_partial[q, :] = ctx^T.T @ W_p_rows

The bias rows (c_attn_b v-slice folded through c_proj_w, plus c_proj_b)
are added on the host during unsharding (they are exactly zero for the
reference setup_inputs). The causal_mask input is the deterministic tril
mask from setup_inputs(); causality is implemented analytically on
device, so the mask tensor itself is unused.
"""

import numpy as np

B, S, D, H = 2, 2048, 1024, 16
HD = D // H  # 64
N_CORES = 8
HPC = 4  # heads per core
GROUPS = 4  # cores per batch
HSL = HPC * HD  # 256: per-core head-column width

_nc_cache = {}


def _build():
    import concourse.bacc as bacc
    import concourse.mybir as mybir
    import concourse.tile as tile

    f32 = mybir.dt.float32
    f16 = mybir.dt.float16

    nc = bacc.Bacc("TRN2", debug=False, num_devices=N_CORES)

    hst = nc.dram_tensor("hst", [D, S], f16, kind="ExternalInput")
    wqkv = nc.dram_tensor("wqkv", [D, 3 * HSL], f16, kind="ExternalInput")
    wp = nc.dram_tensor("wp", [HSL, D], f16, kind="ExternalInput")
    bqk = nc.dram_tensor("bqk", [2 * HSL], f32, kind="ExternalInput")
    outp = nc.dram_tensor("outp", [S, D], f16, kind="ExternalOutput")

    NDT = D // 128  # 8 contraction tiles
    NQB = S // 512  # 4 query blocks
    NKT = S // 128  # 16 key tiles
    EXPSCALE = float(1.0 / np.sqrt(HD))
    EXP = mybir.ActivationFunctionType.Exp
    IDENT = mybir.ActivationFunctionType.Identity

    with tile.TileContext(nc) as tc:
        with (
            tc.tile_pool(name="persist", bufs=1) as persist,
            tc.tile_pool(name="es", bufs=3) as es_pool,
            tc.tile_pool(name="rb", bufs=4) as rb_pool,
            tc.tile_pool(name="ob", bufs=3) as ob_pool,
            tc.tile_pool(name="sc", bufs=2, space="PSUM") as sc_pool,
            tc.tile_pool(name="cx", bufs=2, space="PSUM") as cx_pool,
            tc.tile_pool(name="pj", bufs=2, space="PSUM") as pj_pool,
        ):
            hsT = persist.tile([128, NDT, S], f16)
            wqkv_sb = persist.tile([128, NDT, 3 * HSL], f16)
            wp_sb = persist.tile([128, 2, D], f16)
            bqk_sb = persist.tile([128, 4], f32)
            qkT = persist.tile([128, 4, S], f16)  # ct: 0,1=Q h01/h23, 2,3=K
            vv = persist.tile([128, NKT, HPC * (HD + 1)], f16)  # V aug
            ctxT = persist.tile([128, 2, S], f16)
            tril = persist.tile([128, 128], f16)

            # causal boundary mask: keep where q - p >= 0
            nc.gpsimd.memset(tril, 1.0)
            nc.gpsimd.affine_select(
                out=tril,
                in_=tril,
                compare_op=mybir.AluOpType.is_ge,
                fill=0.0,
                base=0,
                pattern=[[1, 128]],
                channel_multiplier=-1,
            )
            ones_src = persist.tile([128, HPC, 1], f16)
            nc.vector.memset(ones_src, 1.0)
            for kt in range(NKT):
                vcols = vv[:, kt, :].rearrange("p (h c) -> p h c", c=HD + 1)
                nc.vector.tensor_copy(vcols[:, :, HD : HD + 1], ones_src)

            # input DMAs, (dt, nt)-granular in nt-major order
            nc.sync.dma_start(out=bqk_sb, in_=bqk.rearrange("(t p) -> p t", p=128))
            wqkv_r = wqkv.rearrange("(t p) n -> p t n", p=128)
            for dt in range(NDT):
                nc.sync.dma_start(out=wqkv_sb[:, dt, :], in_=wqkv_r[:, dt, :])
            for nt in range(NQB):
                for dt in range(NDT):
                    nc.sync.dma_start(
                        out=hsT[:, dt, nt * 512 : (nt + 1) * 512],
                        in_=hst[
                            dt * 128 : (dt + 1) * 128, nt * 512 : (nt + 1) * 512
                        ],
                    )
            nc.sync.dma_start(out=wp_sb, in_=wp.rearrange("(t p) n -> p t n", p=128))

            def emit_proj_chunk(nt):
                # Q^T/K^T columns [512*nt, 512*(nt+1)) for all 4 ct tiles
                for ct in range(4):
                    pj = pj_pool.tile([128, 512], f32, tag="pj", name=f"pj{nt}_{ct}")
                    for dt in range(NDT):
                        nc.tensor.matmul(
                            pj,
                            wqkv_sb[:, dt, ct * 128 : (ct + 1) * 128],
                            hsT[:, dt, nt * 512 : (nt + 1) * 512],
                            start=(dt == 0),
                            stop=(dt == NDT - 1),
                        )
                    nc.scalar.activation(
                        qkT[:, ct, nt * 512 : (nt + 1) * 512],
                        pj,
                        IDENT,
                        bias=bqk_sb[:, ct : ct + 1],
                    )
                # V rows [512*nt, 512*(nt+1))
                for rt in range(4 * nt, 4 * nt + 4):
                    pvf = pj_pool.tile([128, 512], f32, tag="pj", name=f"pv{rt}")
                    pv = pvf[:, :HSL]
                    for dt in range(NDT):
                        nc.tensor.matmul(
                            pv,
                            hsT[:, dt, rt * 128 : (rt + 1) * 128],
                            wqkv_sb[:, dt, 2 * HSL : 3 * HSL],
                            start=(dt == 0),
                            stop=(dt == NDT - 1),
                        )
                    vtgt = vv[:, rt, :].rearrange("p (h c) -> p h c", c=HD + 1)
                    nc.vector.tensor_copy(
                        vtgt[:, :, 0:HD], pv.rearrange("p (h c) -> p h c", c=HD)
                    )

            def emit_attn_qb(qb, hps=(0, 1)):
                kmax = 4 * (qb + 1)
                for hp in hps:
                    cxa = cx_pool.tile([65, 512], f32, tag="cx", name=f"cxa{qb}_{hp}")
                    cxb = cx_pool.tile([65, 512], f32, tag="cx", name=f"cxb{qb}_{hp}")
                    cxs = (cxa, cxb)

                    def emit_av(kt, es3, off):
                        for hh in range(2):
                            h = 2 * hp + hh
                            nc.tensor.matmul(
                                cxs[hh][:, off:512],
                                vv[:, kt, h * (HD + 1) : (h + 1) * (HD + 1)],
                                es3[:, hh, off:512],
                                start=(kt == 0),
                                stop=(kt == kmax - 1),
                                skip_group_check=True,
                            )

                    pend = None  # AV runs one k-tile behind scores/exp
                    for kt in range(kmax):
                        j = kt - 4 * qb  # >= 0 on the diagonal tiles
                        off = 128 * j if j > 0 else 0
                        scp = sc_pool.tile(
                            [128, 1024], f32, tag="sc", name=f"sc{qb}_{hp}_{kt}"
                        )
                        sc3 = scp.rearrange("p (h c) -> p h c", c=512)
                        for hh in range(2):
                            nc.tensor.matmul(
                                scp[:, hh * 512 + off : (hh + 1) * 512],
                                qkT[
                                    hh * 64 : (hh + 1) * 64,
                                    2 + hp,
                                    kt * 128 : (kt + 1) * 128,
                                ],
                                qkT[
                                    hh * 64 : (hh + 1) * 64,
                                    hp,
                                    qb * 512 + off : (qb + 1) * 512,
                                ],
                                start=True,
                                stop=True,
                                tile_position=(hh * 64, 0),
                            )
                        es = es_pool.tile([128, 1024], f16, tag="es")
                        es3 = es.rearrange("p (h c) -> p h c", c=512)
                        nc.scalar.activation(
                            es3[:, :, off:512],
                            sc3[:, :, off:512],
                            EXP,
                            scale=EXPSCALE,
                        )
                        if j >= 0:  # mask the 128-wide boundary strip
                            for hh in range(2):
                                nc.vector.tensor_mul(
                                    es3[:, hh, off : off + 128],
                                    es3[:, hh, off : off + 128],
                                    tril,
                                )
                        if pend is not None:
                            emit_av(*pend)
                        pend = (kt, es3, off)
                    emit_av(*pend)

                    for hh in range(2):
                        # reciprocal_approx_fast misreads PSUM sources on HW
                        # (integer-ALU seed path); stage the denom row first
                        dstage = rb_pool.tile([1, 512], f32, tag="dst")
                        nc.vector.tensor_copy(dstage, cxs[hh][64:65, :])
                        rec = rb_pool.tile([1, 512], f32, tag="rec")
                        nc.vector.reciprocal_approx_fast(rec, dstage)
                        rbt = rb_pool.tile([64, 512], f32, tag="rbt")
                        nc.gpsimd.partition_broadcast(rbt, rec)
                        nc.vector.tensor_mul(
                            ctxT[
                                hh * 64 : (hh + 1) * 64, hp, qb * 512 : (qb + 1) * 512
                            ],
                            cxs[hh][0:64, :],
                            rbt,
                        )

            def emit_outproj_qb(qb):
                for mt in range(4 * qb, 4 * qb + 4):
                    po0 = pj_pool.tile([128, 512], f32, tag="pj", name=f"po0_{mt}")
                    po1 = pj_pool.tile([128, 512], f32, tag="pj", name=f"po1_{mt}")
                    pos = (po0, po1)
                    for ht in range(2):
                        for et in range(2):
                            nc.tensor.matmul(
                                pos[et],
                                ctxT[:, ht, mt * 128 : (mt + 1) * 128],
                                wp_sb[:, ht, et * 512 : (et + 1) * 512],
                                start=(ht == 0),
                                stop=(ht == 1),
                            )
                    ob = ob_pool.tile([128, 1024], f16, tag="ob")
                    nc.vector.tensor_copy(ob[:, 0:512], pos[0])
                    nc.vector.tensor_copy(ob[:, 512:1024], pos[1])
                    nc.sync.dma_start(
                        out=outp[mt * 128 : (mt + 1) * 128, :], in_=ob
                    )

            emit_proj_chunk(0)
            emit_attn_qb(0)
            emit_proj_chunk(1)
            emit_attn_qb(1)
            emit_proj_chunk(2)
            emit_attn_qb(2)
            emit_proj_chunk(3)
            emit_outproj_qb(0)
            emit_attn_qb(3, hps=(0,))
            emit_outproj_qb(1)
            emit_attn_qb(3, hps=(1,))
            emit_outproj_qb(2)
            emit_outproj_qb(3)

    nc.compile()
    return nc


def build_kernel(*_args, **_kwargs):
    if "k" not in _nc_cache:
        _nc_cache["k"] = _build()
    return _nc_cache["k"]


def make_in_maps(
    hidden_states, c_attn_w, c_attn_b, c_proj_w, c_proj_b, **_unused
):
    hidden_states = np.asarray(hidden_states, dtype=np.float32)
    c_attn_w = np.asarray(c_attn_w, dtype=np.float32)
    c_attn_b = np.asarray(c_attn_b, dtype=np.float32)
    c_proj_w = np.asarray(c_proj_w, dtype=np.float32)
    c_proj_b = np.asarray(c_proj_b, dtype=np.float32)

    in_maps = []
    for c in range(N_CORES):
        b, g = divmod(c, GROUPS)
        cs = slice(g * HSL, (g + 1) * HSL)
        wq = c_attn_w[:, g * HSL : (g + 1) * HSL]
        wk = c_attn_w[:, D + g * HSL : D + (g + 1) * HSL]
        wv = c_attn_w[:, 2 * D + g * HSL : 2 * D + (g + 1) * HSL]
        bq = c_attn_b[g * HSL : (g + 1) * HSL]
        bk = c_attn_b[D + g * HSL : D + (g + 1) * HSL]
        bv = c_attn_b[2 * D + g * HSL : 2 * D + (g + 1) * HSL]
        wps = c_proj_w[cs, :]
        rr = bv.astype(np.float64) @ wps.astype(np.float64)
        if g == 0:
            rr = rr + c_proj_b
        in_maps.append(
            {
                "hst": np.ascontiguousarray(
                    hidden_states[b].T.astype(np.float16)
                ),
                "wqkv": np.ascontiguousarray(
                    np.concatenate([wq, wk, wv], axis=1).astype(np.float16)
                ),
                "wp": np.ascontiguousarray(wps.astype(np.float16)),
                "bqk": np.ascontiguousarray(
                    np.concatenate([bq, bk]).astype(np.float32)
                ),
                "_rrow": np.ascontiguousarray(rr.astype(np.float32)),
            }
        )
    return in_maps


def kernel(
    hidden_states,
    c_attn_w,
    c_attn_b,
    c_proj_w,
    c_proj_b,
    causal_mask=None,
    **_unused,
):
    from concourse.bass_utils import run_bass_kernel_spmd

    nc = build_kernel()
    in_maps = make_in_maps(
        hidden_states, c_attn_w, c_attn_b, c_proj_w, c_proj_b
    )
    rrows = [m.pop("_rrow") for m in in_maps]
    res = run_bass_kernel_spmd(nc, in_maps, list(range(N_CORES)))
    out = np.zeros((B, S, D), dtype=np.float32)
    for c in range(N_CORES):
        out[c // GROUPS] += res.results[c]["outp"] + rrows[c]
    return out
